# revision 40
# baseline (speedup 1.0000x reference)
"""DilateAttention3D (3x3x3 window, dil=1) Trainium2 Bass kernel, 8-core SPMD.

Sharding: core = (b, dc) for b in {0,1}, dc in {0..3}: one batch element and a
D-chunk of 4 planes (halo 1 from zero-padded k/v) per core.

v4 design (dz-pair tiles, host-shingled k, binary mask on DVE):
 - Per-core tile = (dzp, y, xh): 2 D-planes x 6 heads x 16 x-positions =
   192 query columns; key union = 4 dz' x 3 y' x 18 x' = 216 keys, split in
   two x-halves of K2=108 (keys on PSUM partitions).
 - k is shipped pre-shingled from host: [96ch, dzp, y, xh, c, 108] bf16 so
   each QK matmul takes lhsT directly as a contiguous [96,108] SBUF slice
   (the BIR verifier only allows one free dim on the stationary operand).
 - q ships pre-block-diagonalized [NG, 96, TB, 2, 96] bf16 (engine SBUF
   access needs 32-aligned partition starts, so the 16-row per-head copies
   cannot be done on-chip).
 - scores -> ACT exp(0.25 x) -> bf16, then DVE multiplies by a constant
   binary in-window mask [108, 2, 192] (replaces the -BIG rank-17 SVD trick;
   zero-padded boundary keys keep weight exp(0)=1 to match the reference).
 - AV: pa[97(ch|denom), 192q] += vt[:,i,c,:]^T @ amt[:,c,:] over the two
   halves; vt is host-pre-gathered v windows + ones column (denominator).
 - Raw pa is DMA'd out in bf16; the host extracts per-head diagonal blocks
   and divides by the denominator.

Per 2-tile: Pool 6 copies | PE 2 QK + 2 AV | ACT 1 exp | DVE 1 mask + 1 cast
"""
import os
import numpy as np
import ml_dtypes

BF16 = ml_dtypes.bfloat16
B, d, D, H, W = 2, 96, 16, 32, 32
NH, HD = 6, 16
DL, SLAB = 4, 6
TB = 8
NG = 16          # DMA groups of TB 2-tiles
K2 = 108         # keys per x-half (4 dz' x 3 y' x 9 x')
Q2 = 192         # query cols (2 t' x 6 h x 16 x)
NQB = 6          # rotating block-diagonal q buffers

_cache = {}


BIGP = 240.0     # pre-scale mask bias; effective +60 after ACT scale=0.25


def _mask_factors():
    """Exact rank-32 factorization of BIGP*mask[108, 192] per x-half.

    mask[(dz',y',x'), (t',h,x)] = D[dz',t'] * B[9c+x', x]; with inner index
    j=(t'',x''): L[c][key, j] = BIGP*D[dz',t'']*B[9c+x', x''] (values 0/BIGP),
    R[j, col] = delta(t''=t', x''=x) (values 0/1) -> L@R == BIGP*mask exactly,
    and every entry is bf16-exact so the fold adds zero rounding error.
    An additive +BIGP at in-window slots is equivalent to -BIGP outside
    (a per-query constant cancels in softmax); +BIGP keeps rank at 32 <= 128-96.
    """
    band = np.zeros((18, 16), np.float32)
    for x in range(16):
        band[x:x + 3, x] = 1.0
    Dm = np.zeros((4, 2), np.float32)
    for dz in range(4):
        for tp in range(2):
            Dm[dz, tp] = 1.0 if tp <= dz <= tp + 2 else 0.0
    L = np.zeros((2, K2, 32), np.float32)
    for c in range(2):
        for dz in range(4):
            for yp in range(3):
                for xp in range(9):
                    row = (dz * 3 + yp) * 9 + xp
                    L[c, row, 0:16] = BIGP * Dm[dz, 0] * band[9 * c + xp]
                    L[c, row, 16:32] = BIGP * Dm[dz, 1] * band[9 * c + xp]
    R = np.zeros((32, 2, 96), np.float32)
    for tpp in range(2):
        for xpp in range(16):
            for h in range(NH):
                R[tpp * 16 + xpp, tpp, 16 * h + xpp] = 1.0
    return L, R


def _build_nc():
    from concourse import bacc, mybir
    import concourse.tile as tile
    from contextlib import ExitStack

    f32 = mybir.dt.float32
    bf16 = mybir.dt.bfloat16
    nc = bacc.Bacc(None, target_bir_lowering=False, debug=True)

    qb_d = nc.declare_dram_parameter("qb", [NG, 96, TB, 2, 96], bf16,
                                     isOutput=False)
    rc_d = nc.declare_dram_parameter("rc", [32, TB, 2, 96], bf16,
                                     isOutput=False)
    ksh_d = nc.declare_dram_parameter("ksh", [128, 2, H, 2, 2, K2], bf16,
                                      isOutput=False)
    vt_d = nc.declare_dram_parameter("vt", [NG, K2, TB, 2, 97], bf16,
                                     isOutput=False)
    # out split into 768B/partition slices over 96 partitions plus a
    # 1-partition denominator plane: wider d2h DMAs all landed on one SDMA
    # engine and serialized.
    out_d = nc.declare_dram_parameter("out", [NG, 2, 2, 96, TB // 2, 96], bf16,
                                      isOutput=True)
    den_d = nc.declare_dram_parameter("den", [NG, TB, Q2], bf16, isOutput=True)

    with ExitStack() as ctx:
        tc = ctx.enter_context(tile.TileContext(nc))
        cpool = ctx.enter_context(tc.tile_pool(name="consts", bufs=1))
        vpool = ctx.enter_context(tc.tile_pool(name="vt", bufs=4))
        epool = ctx.enter_context(tc.tile_pool(name="amt", bufs=4))
        opool = ctx.enter_context(tc.tile_pool(name="o", bufs=4))
        pspool = ctx.enter_context(tc.tile_pool(name="ps", bufs=4, space="PSUM"))
        papool = ctx.enter_context(tc.tile_pool(name="pa", bufs=4, space="PSUM"))

        ksh_sb = cpool.tile([128, 2, H, 2, 2, K2], bf16)
        nc.sync.dma_start(ksh_sb[:, 0], ksh_d[:, 0])

        qb_bufs = []
        for i in range(NQB):
            qbb = cpool.tile([128, TB, 2, 96], bf16, tag=f"qbbuf{i}",
                             name=f"qbbuf{i}")
            nc.sync.dma_start(qbb[96:128], rc_d[:])
            qb_bufs.append(qbb)
        den_all = cpool.tile([1, NG, TB, Q2], bf16, name="den_all")

        for g_ in range(NG):
            dzp, yq = divmod(g_, 8)
            if 1 <= g_ <= 4:
                # stream the dzp=1 half of ksh in y-chunks behind the early
                # dzp=0 groups so the first QK doesn't wait on all of ksh
                yc = g_ - 1
                nc.sync.dma_start(ksh_sb[:, 1, 8 * yc:8 * yc + 8],
                                  ksh_d[:, 1, 8 * yc:8 * yc + 8])
            qb4 = qb_bufs[g_ % NQB]
            nc.gpsimd.dma_start(qb4[0:96], qb_d[g_])
            vt4 = vpool.tile([K2, TB, 2, 97], bf16, tag="vt4")
            nc.sync.dma_start(vt4[:], vt_d[g_])
            obh = opool.tile([96, 2, TB, 96], bf16, tag="obh")

            for i in range(TB):
                dy, xh = divmod(i, 2)
                y = 4 * yq + dy

                ps = pspool.tile([K2, 2, Q2], f32, tag="ps")
                for c in range(2):
                    nc.tensor.matmul(
                        ps[:, c, :], lhsT=ksh_sb[:, dzp, y, xh, c, :],
                        rhs=qb4[:, i, :, :], start=True, stop=True,
                    )
                amt = epool.tile([K2, 2, Q2], bf16, tag="amt")
                nc.scalar.activation(
                    amt[:], ps[:], mybir.ActivationFunctionType.Exp, scale=0.25
                )

                pa = papool.tile([97, Q2], f32, tag="pa")
                for c in range(2):
                    nc.tensor.matmul(
                        pa[:], lhsT=vt4[:, i, c, :], rhs=amt[:, c, :],
                        start=(c == 0), stop=(c == 1),
                    )
                nc.vector.tensor_copy(obh[:, :, i, :], pa[0:96, :])
                nc.vector.tensor_copy(den_all[:, g_, i, :], pa[96:97, :])
            for tp in range(2):
                for dyh in range(2):
                    nc.sync.dma_start(
                        out_d[g_, tp, dyh],
                        obh[:, tp, 4 * dyh:4 * dyh + 4, :])
        nc.sync.dma_start(den_d[:], den_all[0:1])
    nc.compile()
    return nc


def _host_prep(q, k, v, b, dc):
    kp = np.pad(k[b], ((0, 0), (1, 1), (1, 1), (1, 1)))
    vp = np.pad(v[b], ((0, 0), (1, 1), (1, 1), (1, 1)))
    k_slab = kp[:, 4 * dc:4 * dc + SLAB]          # [96,6,34,34]
    v_slab = vp[:, 4 * dc:4 * dc + SLAB]
    qs = q[b][:, 4 * dc:4 * dc + DL]              # [96,4,32,32]

    # block-diagonal q: [g, ch, i, t', qcol] with head h in rows/cols 16h..
    qblk = np.zeros((NG, 96, TB, 2, 96), np.float32)
    # qs -> [ch, dzp, t', yq, dy, xh, x]
    qr = qs.reshape(96, 2, 2, 8, 4, 2, 16)
    for h in range(NH):
        blk = qr[16 * h:16 * h + 16]              # [16, dzp, t', yq, dy, xh, x]
        # -> [g=(dzp,yq), c', i=(dy,xh), t', x]
        blk = blk.transpose(1, 3, 0, 4, 5, 2, 6).reshape(NG, 16, TB, 2, 16)
        qblk[:, 16 * h:16 * h + 16, :, :, 16 * h:16 * h + 16] = blk

    def shingle(slab):
        sw = np.lib.stride_tricks.sliding_window_view(slab, (4, 3, 18),
                                                      axis=(1, 2, 3))
        sw = sw[:, ::2, :, ::16]                  # [96, 2, 32, 2, 4, 3, 18]
        sw = sw.reshape(96, 2, H, 2, 4, 3, 2, 9)
        # -> [ch, dzp, y, xh, c, (dz',y',x')=108]
        return sw.transpose(0, 1, 2, 3, 6, 4, 5, 7).reshape(96, 2, H, 2, 2, K2)

    L, _ = _mask_factors()
    ksh = np.zeros((128, 2, H, 2, 2, K2), np.float32)
    ksh[0:96] = shingle(k_slab)
    for c in range(2):
        ksh[96:128, :, :, :, c, :] = L[c].T[:, None, None, None, :]
    vsh = shingle(v_slab)                         # [96, dzp, y, xh, c, 108]
    vtw = vsh.transpose(1, 2, 3, 4, 5, 0)         # [dzp, y, xh, c, 108, 96]
    vt = np.ones((2, H, 2, 2, K2, 97), np.float32)
    vt[..., :96] = vtw
    vt = vt.reshape(2, 8, 4, 2, 2, K2, 97)        # [dzp, yq, dy, xh, c, row, col]
    vt = vt.transpose(0, 1, 5, 2, 3, 4, 6).reshape(NG, K2, TB, 2, 97)
    return (np.ascontiguousarray(qblk.astype(BF16)),
            np.ascontiguousarray(ksh.astype(BF16)),
            np.ascontiguousarray(vt.astype(BF16)))


def kernel(q, k, v):
    q = np.asarray(q, np.float32)
    k = np.asarray(k, np.float32)
    v = np.asarray(v, np.float32)

    if "nc" not in _cache:
        _cache["nc"] = _build_nc()
    nc = _cache["nc"]

    from concourse.bass_utils import run_bass_kernel_spmd

    _, R = _mask_factors()
    rc = np.ascontiguousarray(
        np.broadcast_to(R[:, None], (32, TB, 2, 96)).astype(BF16))
    in_maps = []
    for core in range(8):
        b, dc = divmod(core, 4)
        qblk, ksh, vt = _host_prep(q, k, v, b, dc)
        in_maps.append({"qb": qblk, "ksh": ksh, "vt": vt, "rc": rc})

    res = run_bass_kernel_spmd(nc, in_maps, list(range(8)),
                               trace=bool(int(os.environ.get("KTRACE", "0"))))
    _cache["last_results"] = res

    hsel = np.arange(NH)
    full = np.zeros((B, D, H, W, d), np.float32)
    for core in range(8):
        b, dc = divmod(core, 4)
        ob = res.results[core]["out"].astype(np.float32)  # [NG,2,2,96,TB/2,96]
        dn = res.results[core]["den"].astype(np.float32)  # [NG, TB, 192]
        # -> [dzp, yq, tp, hr, c', dy, xh, h, x]
        ob = ob.reshape(2, 8, 2, 2, 96, 2, 2, 96)  # [dzp,yq,tp,dyh,ch,dyl,xh,hx]
        ob = ob.transpose(0, 1, 2, 4, 3, 5, 6, 7)  # [dzp,yq,tp,ch,dyh,dyl,xh,hx]
        blocks = ob.reshape(2, 8, 2, NH, 16, 4, 2, NH, 16)
        num = blocks[:, :, :, hsel, :, :, :, hsel]  # [h,dzp,yq,tp,c',dy,xh,x]
        num = num.transpose(1, 3, 2, 5, 6, 7, 0, 4)  # [dzp,tp,yq,dy,xh,x,h,c']
        den = dn.reshape(2, 8, 4, 2, 2, NH, 16)      # [dzp,yq,dy,xh,tp,h,x]
        dd = den.transpose(0, 4, 1, 2, 3, 6, 5)      # [dzp,tp,yq,dy,xh,x,h]
        o = num / dd[..., None]
        full[b, 4 * dc:4 * dc + DL] = o.reshape(DL, H, W, d)
    return full
